# revision 8
# baseline (speedup 1.0000x reference)
"""MoE layer (top-2 of 16 experts) for Trainium2, expert-parallel over 8 cores.

Strategy:
  - Host (numpy): gate matmul + softmax + top-2 routing + combine weights +
    aux loss (0.13% of total FLOPs), token gather per expert.
  - Device (Bass/Tile, SPMD over 8 cores): each core holds 2 experts'
    MLP weights (bf16) and runs  relu(x @ w1 + b1) @ w2  over its
    compacted token batch.  Layout is weights-stationary / tokens-streaming:
    every matmul is lhsT=[128,128] weight block, rhs=[128, TN] token block,
    contraction on the partition dim, accumulated in PSUM f32.
  - Host: scatter-add  cw * (y + b2)  back to the full [T, D] output.
"""

import sys

sys.path.insert(0, "/opt/trn_rl_repo")

import numpy as np
import ml_dtypes

B, S, D = 4, 1600, 768
E, TOPK = 16, 2
H = 3072
T = B * S
NCORES = 8
EPC = E // NCORES          # experts per core = 2
ND, NH = D // 128, H // 128  # 6, 24
TN_MAX = 512               # PSUM bank = 512 f32 -> max moving free dim

_build_cache = {}


def _build(C, TN, ndx):
    """Build the SPMD Bass program for per-expert token capacity C = nt*TN.

    ndx = number of 128-row contraction blocks in layer 1 (ND, or ND+1 when
    b1 is folded in as a ones-row block)."""
    if (C, TN, ndx) in _build_cache:
        return _build_cache[(C, TN, ndx)]
    import concourse.mybir as mybir
    from concourse import bacc
    from concourse.tile import TileContext

    bf16 = mybir.dt.bfloat16
    f32 = mybir.dt.float32
    Relu = mybir.ActivationFunctionType.Relu
    nt = C // TN

    nc = bacc.Bacc(None, target_bir_lowering=False, debug=False)
    xT = nc.dram_tensor("xT", [EPC, ndx, 128, C], bf16, kind="ExternalInput")
    w1l = nc.dram_tensor("w1l", [EPC, NH, ndx, 128, 128], bf16, kind="ExternalInput")
    w2l = nc.dram_tensor("w2l", [EPC, ND, NH, 128, 128], bf16, kind="ExternalInput")
    yT = nc.dram_tensor("yT", [EPC, ND, 128, C], f32, kind="ExternalOutput")

    with TileContext(nc) as tc:
        with (
            tc.tile_pool(name="xp", bufs=2 * ND) as xp,
            tc.tile_pool(name="h1p", bufs=2 * NH) as h1p,
            tc.tile_pool(name="w1p", bufs=3 * ND) as w1p,
            tc.tile_pool(name="w2p", bufs=2 * NH) as w2p,
            tc.tile_pool(name="op", bufs=4) as op,
            tc.tile_pool(name="ps1", bufs=4, space="PSUM") as ps1p,
            tc.tile_pool(name="ps2", bufs=4, space="PSUM") as ps2p,
        ):
            for e in range(EPC):
                xts = []
                for d in range(ndx):
                    xt = xp.tile([128, C], bf16)
                    nc.sync.dma_start(out=xt[:], in_=xT[e, d])
                    xts.append(xt)
                # layer 1: h1T[h, tokens] = relu(w1.T @ xT + b1)
                h1ts = []
                for h in range(NH):
                    w1ts = []
                    for d in range(ndx):
                        wt = w1p.tile([128, 128], bf16)
                        nc.sync.dma_start(out=wt[:], in_=w1l[e, h, d])
                        w1ts.append(wt)
                    h1t = h1p.tile([128, C], bf16)
                    pss = [ps1p.tile([128, TN], f32, tag="ps1", name="ps1t") for _ in range(nt)]
                    for d in range(ndx):
                        for t in range(nt):
                            nc.tensor.matmul(
                                pss[t][:],
                                w1ts[d][:],
                                xts[d][:, t * TN:(t + 1) * TN],
                                start=(d == 0),
                                stop=(d == ndx - 1),
                            )
                    for t in range(nt):
                        nc.scalar.activation(
                            h1t[:, t * TN:(t + 1) * TN], pss[t][:], Relu
                        )
                    h1ts.append(h1t)
                # layer 2: yT[dd, tokens] = w2.T @ h1T
                for dd in range(ND):
                    w2ts = []
                    for h in range(NH):
                        wt = w2p.tile([128, 128], bf16)
                        nc.sync.dma_start(out=wt[:], in_=w2l[e, dd, h])
                        w2ts.append(wt)
                    pss = [ps2p.tile([128, TN], f32, tag="ps2", name="ps2t") for _ in range(nt)]
                    for h in range(NH):
                        for t in range(nt):
                            nc.tensor.matmul(
                                pss[t][:],
                                w2ts[h][:],
                                h1ts[h][:, t * TN:(t + 1) * TN],
                                start=(h == 0),
                                stop=(h == NH - 1),
                            )
                    for t in range(nt):
                        ot = op.tile([128, TN], f32)
                        nc.vector.tensor_copy(ot[:], pss[t][:])
                        nc.sync.dma_start(
                            out=yT[e, dd, :, t * TN:(t + 1) * TN], in_=ot[:]
                        )
    nc.compile()
    _build_cache[(C, TN, ndx)] = nc
    return nc


def kernel(x, gate_w, w1, b1, w2, b2, _trace=False):
    x = np.asarray(x, np.float32)
    gate_w = np.asarray(gate_w, np.float32)
    w1 = np.asarray(w1, np.float32)
    b1 = np.asarray(b1, np.float32)
    w2 = np.asarray(w2, np.float32)
    b2 = np.asarray(b2, np.float32)

    xf = x.reshape(-1, D)

    # ---- host: gate + top-2 routing + aux loss ----
    logits = xf @ gate_w.T                       # [T, E]
    mx = logits.max(axis=1, keepdims=True)
    ex = np.exp(logits - mx)
    probs = ex / ex.sum(axis=1, keepdims=True)   # [T, E]

    part = np.argpartition(-probs, TOPK - 1, axis=1)[:, :TOPK]
    pv = np.take_along_axis(probs, part, axis=1)
    order = np.argsort(-pv, axis=1, kind="stable")
    topi = np.take_along_axis(part, order, axis=1)   # [T, 2] sorted desc
    topv = np.take_along_axis(pv, order, axis=1)
    cwv = topv / topv.sum(axis=1, keepdims=True)     # renormalized weights

    kth = topv[:, -1:]
    mean_prob = probs.mean(axis=0)
    mean_topk = (probs > kth).astype(np.float32).mean(axis=0)
    aux = np.float32((mean_prob * mean_topk).sum() * E)

    # ---- host: per-expert gather ----
    idxs, wts = [], []
    for e in range(E):
        m0 = topi[:, 0] == e
        m1 = topi[:, 1] == e
        idx = np.nonzero(m0 | m1)[0]
        wt = np.where(m0[idx], cwv[idx, 0], cwv[idx, 1]).astype(np.float32)
        idxs.append(idx)
        wts.append(wt)
    maxc = max(1, max(len(i) for i in idxs))
    nt = -(-maxc // TN_MAX)                  # number of token tiles
    TN = -(-(-(-maxc // nt)) // 64) * 64     # ceil(maxc/nt) rounded up to 64
    C = nt * TN

    has_b1 = bool(np.any(b1))
    ndx = ND + 1 if has_b1 else ND
    nc = _build(C, TN, ndx)

    in_maps = []
    for c in range(NCORES):
        ge = list(range(EPC * c, EPC * (c + 1)))
        xTl = np.zeros((EPC, ndx * 128, C), np.float32)
        for j, e in enumerate(ge):
            xTl[j, :D, : len(idxs[e])] = xf[idxs[e]].T
            if has_b1:
                xTl[j, D, : len(idxs[e])] = 1.0  # ones row activates the b1 block
        w1e = w1[ge]
        if has_b1:
            w1pad = np.zeros((EPC, 128, H), np.float32)
            w1pad[:, 0, :] = b1[ge]
            w1e = np.concatenate([w1e, w1pad], axis=1)
        w1blk = w1e.reshape(EPC, ndx, 128, NH, 128).transpose(0, 3, 1, 2, 4)
        # (e, h, p, dd, q) -> (e, dd, h, p, q)
        w2blk = w2[ge].reshape(EPC, NH, 128, ND, 128).transpose(0, 3, 1, 2, 4)
        in_maps.append(
            {
                "xT": np.ascontiguousarray(xTl.reshape(EPC, ndx, 128, C)).astype(
                    ml_dtypes.bfloat16
                ),
                "w1l": np.ascontiguousarray(w1blk).astype(ml_dtypes.bfloat16),
                "w2l": np.ascontiguousarray(w2blk).astype(ml_dtypes.bfloat16),
            }
        )

    from concourse import bass_utils

    try:
        res = bass_utils.run_bass_kernel_spmd(
            nc, in_maps, list(range(NCORES)), trace=_trace
        )
    except Exception:
        if not _trace:
            raise
        res = bass_utils.run_bass_kernel_spmd(nc, in_maps, list(range(NCORES)))
    kernel.last_results = res
    kernel.last_args = (nc, in_maps)

    # ---- host: weighted scatter-add back to full output ----
    out = np.zeros((T, D), np.float32)
    for c in range(NCORES):
        yTl = np.asarray(res.results[c]["yT"], np.float32)  # [EPC, ND, 128, C]
        for j in range(EPC):
            e = EPC * c + j
            cnt = len(idxs[e])
            if cnt == 0:
                continue
            y = yTl[j].reshape(D, C)[:, :cnt].T  # [cnt, D]
            out[idxs[e]] += wts[e][:, None] * (y + b2[e])

    return out.reshape(B, S, D), aux


# revision 10
# speedup vs baseline: 1.0259x; 1.0259x over previous
"""MoE layer (top-2 of 16 experts) for Trainium2, expert-parallel over 8 cores.

Strategy:
  - Host (numpy): gate matmul + softmax + top-2 routing + combine weights +
    aux loss (0.13% of total FLOPs), token gather per expert.
  - Device (Bass/Tile, SPMD over 8 cores): each core holds 2 experts'
    MLP weights (bf16) and runs  relu(x @ w1 + b1) @ w2  over its
    compacted token batch.  Layout is weights-stationary / tokens-streaming:
    every matmul is lhsT=[128,128] weight block, rhs=[128, TN] token block,
    contraction on the partition dim, accumulated in PSUM f32.
  - Host: scatter-add  cw * (y + b2)  back to the full [T, D] output.
"""

import sys

sys.path.insert(0, "/opt/trn_rl_repo")

import numpy as np
import ml_dtypes

B, S, D = 4, 1600, 768
E, TOPK = 16, 2
H = 3072
T = B * S
NCORES = 8
EPC = E // NCORES          # experts per core = 2
ND, NH = D // 128, H // 128  # 6, 24
TN_MAX = 512               # PSUM bank = 512 f32 -> max moving free dim

_build_cache = {}


def _build(C, TN, ndx):
    """Build the SPMD Bass program for per-expert token capacity C = nt*TN.

    ndx = number of 128-row contraction blocks in layer 1 (ND, or ND+1 when
    b1 is folded in as a ones-row block)."""
    if (C, TN, ndx) in _build_cache:
        return _build_cache[(C, TN, ndx)]
    import concourse.mybir as mybir
    from concourse import bacc
    from concourse.tile import TileContext

    bf16 = mybir.dt.bfloat16
    f32 = mybir.dt.float32
    Relu = mybir.ActivationFunctionType.Relu
    nt = C // TN

    nc = bacc.Bacc(None, target_bir_lowering=False, debug=False)
    xT = nc.dram_tensor("xT", [EPC, ndx, 128, C], bf16, kind="ExternalInput")
    w1l = nc.dram_tensor("w1l", [EPC, NH, ndx, 128, 128], bf16, kind="ExternalInput")
    w2l = nc.dram_tensor("w2l", [EPC, ND, NH, 128, 128], bf16, kind="ExternalInput")
    yT = nc.dram_tensor("yT", [EPC, ND, 128, C], f32, kind="ExternalOutput")

    with TileContext(nc) as tc:
        with (
            tc.tile_pool(name="xp", bufs=2 * ND) as xp,
            tc.tile_pool(name="h1p", bufs=2 * NH) as h1p,
            tc.tile_pool(name="w1p", bufs=3 * ND) as w1p,
            tc.tile_pool(name="w2p", bufs=2 * NH) as w2p,
            tc.tile_pool(name="op", bufs=4) as op,
            tc.tile_pool(name="ps1", bufs=4, space="PSUM") as ps1p,
            tc.tile_pool(name="ps2", bufs=4, space="PSUM") as ps2p,
        ):
            for e in range(EPC):
                xts = []
                for d in range(ndx):
                    xt = xp.tile([128, C], bf16)
                    nc.gpsimd.dma_start(out=xt[:], in_=xT[e, d])
                    xts.append(xt)
                # layer 1: h1T[h, tokens] = relu(w1.T @ xT + b1)
                h1ts = []
                for h in range(NH):
                    w1ts = []
                    for d in range(ndx):
                        wt = w1p.tile([128, 128], bf16)
                        nc.sync.dma_start(out=wt[:], in_=w1l[e, h, d])
                        w1ts.append(wt)
                    h1t = h1p.tile([128, C], bf16)
                    pss = [ps1p.tile([128, TN], f32, tag="ps1", name="ps1t") for _ in range(nt)]
                    for d in range(ndx):
                        for t in range(nt):
                            nc.tensor.matmul(
                                pss[t][:],
                                w1ts[d][:],
                                xts[d][:, t * TN:(t + 1) * TN],
                                start=(d == 0),
                                stop=(d == ndx - 1),
                            )
                    for t in range(nt):
                        nc.scalar.activation(
                            h1t[:, t * TN:(t + 1) * TN], pss[t][:], Relu
                        )
                    h1ts.append(h1t)
                # layer 2: yT[dd, tokens] = w2.T @ h1T
                for dd in range(ND):
                    w2ts = []
                    for h in range(NH):
                        wt = w2p.tile([128, 128], bf16)
                        nc.sync.dma_start(out=wt[:], in_=w2l[e, dd, h])
                        w2ts.append(wt)
                    pss = [ps2p.tile([128, TN], f32, tag="ps2", name="ps2t") for _ in range(nt)]
                    for h in range(NH):
                        for t in range(nt):
                            nc.tensor.matmul(
                                pss[t][:],
                                w2ts[h][:],
                                h1ts[h][:, t * TN:(t + 1) * TN],
                                start=(h == 0),
                                stop=(h == NH - 1),
                            )
                    for t in range(nt):
                        ot = op.tile([128, TN], f32)
                        nc.vector.tensor_copy(ot[:], pss[t][:])
                        nc.scalar.dma_start(
                            out=yT[e, dd, :, t * TN:(t + 1) * TN], in_=ot[:]
                        )
    nc.compile()
    _build_cache[(C, TN, ndx)] = nc
    return nc


def kernel(x, gate_w, w1, b1, w2, b2, _trace=False):
    x = np.asarray(x, np.float32)
    gate_w = np.asarray(gate_w, np.float32)
    w1 = np.asarray(w1, np.float32)
    b1 = np.asarray(b1, np.float32)
    w2 = np.asarray(w2, np.float32)
    b2 = np.asarray(b2, np.float32)

    xf = x.reshape(-1, D)

    # ---- host: gate + top-2 routing + aux loss ----
    logits = xf @ gate_w.T                       # [T, E]
    mx = logits.max(axis=1, keepdims=True)
    ex = np.exp(logits - mx)
    probs = ex / ex.sum(axis=1, keepdims=True)   # [T, E]

    part = np.argpartition(-probs, TOPK - 1, axis=1)[:, :TOPK]
    pv = np.take_along_axis(probs, part, axis=1)
    order = np.argsort(-pv, axis=1, kind="stable")
    topi = np.take_along_axis(part, order, axis=1)   # [T, 2] sorted desc
    topv = np.take_along_axis(pv, order, axis=1)
    cwv = topv / topv.sum(axis=1, keepdims=True)     # renormalized weights

    kth = topv[:, -1:]
    mean_prob = probs.mean(axis=0)
    mean_topk = (probs > kth).astype(np.float32).mean(axis=0)
    aux = np.float32((mean_prob * mean_topk).sum() * E)

    # ---- host: per-expert gather ----
    idxs, wts = [], []
    for e in range(E):
        m0 = topi[:, 0] == e
        m1 = topi[:, 1] == e
        idx = np.nonzero(m0 | m1)[0]
        wt = np.where(m0[idx], cwv[idx, 0], cwv[idx, 1]).astype(np.float32)
        idxs.append(idx)
        wts.append(wt)
    maxc = max(1, max(len(i) for i in idxs))
    nt = -(-maxc // TN_MAX)                  # number of token tiles
    TN = -(-(-(-maxc // nt)) // 64) * 64     # ceil(maxc/nt) rounded up to 64
    C = nt * TN

    has_b1 = bool(np.any(b1))
    ndx = ND + 1 if has_b1 else ND
    nc = _build(C, TN, ndx)

    in_maps = []
    for c in range(NCORES):
        ge = list(range(EPC * c, EPC * (c + 1)))
        xTl = np.zeros((EPC, ndx * 128, C), np.float32)
        for j, e in enumerate(ge):
            xTl[j, :D, : len(idxs[e])] = xf[idxs[e]].T
            if has_b1:
                xTl[j, D, : len(idxs[e])] = 1.0  # ones row activates the b1 block
        w1e = w1[ge]
        if has_b1:
            w1pad = np.zeros((EPC, 128, H), np.float32)
            w1pad[:, 0, :] = b1[ge]
            w1e = np.concatenate([w1e, w1pad], axis=1)
        w1blk = w1e.reshape(EPC, ndx, 128, NH, 128).transpose(0, 3, 1, 2, 4)
        # (e, h, p, dd, q) -> (e, dd, h, p, q)
        w2blk = w2[ge].reshape(EPC, NH, 128, ND, 128).transpose(0, 3, 1, 2, 4)
        in_maps.append(
            {
                "xT": np.ascontiguousarray(xTl.reshape(EPC, ndx, 128, C)).astype(
                    ml_dtypes.bfloat16
                ),
                "w1l": np.ascontiguousarray(w1blk).astype(ml_dtypes.bfloat16),
                "w2l": np.ascontiguousarray(w2blk).astype(ml_dtypes.bfloat16),
            }
        )

    from concourse import bass_utils

    try:
        res = bass_utils.run_bass_kernel_spmd(
            nc, in_maps, list(range(NCORES)), trace=_trace
        )
    except Exception:
        if not _trace:
            raise
        res = bass_utils.run_bass_kernel_spmd(nc, in_maps, list(range(NCORES)))
    kernel.last_results = res
    kernel.last_args = (nc, in_maps)

    # ---- host: weighted scatter-add back to full output ----
    out = np.zeros((T, D), np.float32)
    for c in range(NCORES):
        yTl = np.asarray(res.results[c]["yT"], np.float32)  # [EPC, ND, 128, C]
        for j in range(EPC):
            e = EPC * c + j
            cnt = len(idxs[e])
            if cnt == 0:
                continue
            y = yTl[j].reshape(D, C)[:, :cnt].T  # [cnt, D]
            out[idxs[e]] += wts[e][:, None] * (y + b2[e])

    return out.reshape(B, S, D), aux
